# revision 28
# baseline (speedup 1.0000x reference)
"""DeepSeekV3-style MoE block on 8 Trainium2 NeuronCores.

Strategy (expert-parallel, host-routed dispatch/combine):
  - Host computes the (tiny) sigmoid gate in fp32 numpy, does top-2 selection
    and builds per-expert token lists (the "all-to-all dispatch" happens while
    sharding the inputs).
  - Core e runs expert e's SwiGLU over its gathered tokens (padded to a fixed
    capacity) plus a 1/8 token-slice of the shared expert, all in bf16 on the
    TensorEngine with fp32 PSUM accumulation.  Gate scaling is applied on-chip.
  - The host scatter-adds the per-core outputs back into the full [B,S,H]
    tensor (the "combine" happens while unsharding).

All matmuls are laid out so no on-chip transposes are needed:
  phase A:  act[f,c] = silu(w1[h,f].T @ x[h,c]) * (w3[h,f].T @ x[h,c])
  phase B:  y[c,h]   = act[f,c].T @ w2[f,h]    (scaled by the gate weight)
Host-side pre-tiling puts every DRAM operand in [128, ...] partition-major
layout so each DMA is contiguous.
"""

import hashlib
import os
import pickle
import sys

for _p in ("/opt/trn_rl_repo", "/opt/pypackages"):
    if _p not in sys.path:
        sys.path.append(_p)

from contextlib import ExitStack

import numpy as np
import ml_dtypes

import concourse.bacc as bacc
import concourse.mybir as mybir
import concourse.tile as tile
from concourse import bass2jax
from concourse.bass_utils import run_bass_kernel_spmd

_NEFF_CACHE_DIR = os.path.expanduser("~/.cache/bass_neff_cache")


def _install_neff_cache():
    """Persist compiled bass_exec NEFFs across processes (walrus takes
    minutes for this kernel; the result is a pure function of the HLO)."""
    if getattr(bass2jax, "_ant_neff_cache_wrapped", False):
        return
    inner = bass2jax.neuronx_cc_hook

    def cached_hook(code, code_format, platform_version, file_prefix):
        c = code if isinstance(code, (bytes, bytearray)) else str(code).encode()
        if b"bass_exec" not in c:
            return inner(code, code_format, platform_version, file_prefix)
        key = hashlib.sha256(bytes(c)).hexdigest()
        path = os.path.join(_NEFF_CACHE_DIR, key + ".pkl")
        try:
            if os.path.exists(path):
                with open(path, "rb") as f:
                    return pickle.load(f)
        except Exception:
            pass
        r = inner(code, code_format, platform_version, file_prefix)
        try:
            os.makedirs(_NEFF_CACHE_DIR, exist_ok=True)
            tmp = f"{path}.tmp{os.getpid()}"
            with open(tmp, "wb") as f:
                pickle.dump(r, f)
            os.replace(tmp, path)
        except Exception:
            pass
        return r

    bass2jax.neuronx_cc_hook = cached_hook
    bass2jax._ant_neff_cache_wrapped = True


_install_neff_cache()

BF16 = ml_dtypes.bfloat16
P = 128
H = 2048
F = 1408
E = 8
TOPK = 2
NCORES = 8
KH = H // P   # 16 contraction tiles over H
KF = F // P   # 11 contraction tiles over F
HB = H // 512  # 4 output column blocks

FP32 = mybir.dt.float32
BF16_DT = mybir.dt.bfloat16


def _chunks(C):
    """Split C (multiple of 128) into 512-wide chunks (+ remainder).
    N=512 matmuls amortize the PE sequencer's ~165ns/instruction dispatch
    cost; narrower chunks go sequencer-bound."""
    out = []
    c0 = 0
    while c0 < C:
        cb = min(512, C - c0)
        out.append((c0, cb))
        c0 += cb
    return out


CFG = {
    "w13_split": 2,   # dma_starts per w1f/w3f tile
    "w13_split0": 1,  # split for the startup-critical f=0 tiles
    "w13_bufs": 4,
    "x_split": 1,     # dma_starts per x remainder piece
    "x_split0": 1,    # dma_starts per x first-chunk piece
    "w2_split": 1,    # dma_starts per w2 f-slice
    "w2_defer_f": 2,  # emit the w2 bulk load at this f iteration
    "out_split": 1,   # dma_starts per output tile
    "ps1_bufs": 2,
    "ps2_bufs": 4,
    "o_bufs": 6,
    "silu_bufs": 3,
    "dma_eng": "sync",  # w13 weight stream issue engine
    "x_eng": "sync",    # x load issue engine
    "w2_eng": "sync",   # bulk w2 load issue engine
    "out_eng": "sync",  # output store issue engine
}


def _split_dma(eng, dst, src, n):
    w = dst.shape[-1]
    step = -(-w // n)
    for i in range(0, w, step):
        j = min(w, i + step)
        eng.dma_start(dst[:, i:j], src[:, i:j])


def _build(nc, C_r, C_s):
    """Emit the per-core program: routed expert (C_r tokens, gated) then the
    shared-expert slice (C_s tokens)."""
    dram = {}
    for name, shape, dt in [
        ("xr", [P, KH * C_r], BF16_DT),
        ("gr", [P, -(-C_r // P)], FP32),
        ("w1", [P, KF * KH * P], BF16_DT),
        ("w3", [P, KF * KH * P], BF16_DT),
        ("w2", [P, KF * H], BF16_DT),
        ("xs", [P, KH * C_s], BF16_DT),
        ("s1", [P, KF * KH * P], BF16_DT),
        ("s3", [P, KF * KH * P], BF16_DT),
        ("s2", [P, KF * H], BF16_DT),
    ]:
        dram[name] = nc.dram_tensor(name, shape, dt, kind="ExternalInput")
    yr = nc.dram_tensor("yr", [C_r, H], FP32, kind="ExternalOutput")
    ys = nc.dram_tensor("ys", [C_s, H], FP32, kind="ExternalOutput")

    with tile.TileContext(nc) as tc, ExitStack() as ctx:
        pool = ctx.enter_context(tc.tile_pool(name="main", bufs=1))
        psum = ctx.enter_context(tc.tile_pool(name="ps", bufs=1, space="PSUM"))
        dmae = getattr(nc, CFG["dma_eng"])
        xeng = getattr(nc, CFG["x_eng"])
        w2eng = getattr(nc, CFG["w2_eng"])
        oeng = getattr(nc, CFG["out_eng"])

        def problem(tag, xd, w1d, w3d, w2d, yd, C, gd=None):
            # resident x: [128, KH*C]; DMA per (chunk, contraction-tile) so the
            # first chunk's columns land before anything else
            x_sb = pool.tile([P, KH * C], BF16_DT, tag=f"x_{tag}", bufs=1)
            g_sb = None
            if gd is not None:
                g_sb = pool.tile([P, -(-C // P)], FP32, tag=f"g_{tag}", bufs=1)
                nc.sync.dma_start(g_sb[:], gd[:])

            w2_sb = pool.tile([P, KF * H], BF16_DT, tag="w2", bufs=1)
            act_sb = pool.tile([P, KF * C], BF16_DT, tag=f"act_{tag}", bufs=1)

            # ---- phase A: act[f, c] = silu(x@w1.T) * (x@w3.T), [F, C] layout
            chunks = _chunks(C)
            for f in range(KF):
                wsplit = CFG["w13_split0"] if f == 0 else CFG["w13_split"]
                w1f = pool.tile([P, KH * P], BF16_DT, tag="w1f", bufs=CFG["w13_bufs"])
                _split_dma(
                    dmae, w1f[:], w1d[:, f * KH * P : (f + 1) * KH * P], wsplit
                )
                w3f = pool.tile([P, KH * P], BF16_DT, tag="w3f", bufs=CFG["w13_bufs"])
                _split_dma(
                    dmae, w3f[:], w3d[:, f * KH * P : (f + 1) * KH * P], wsplit
                )
                if f == CFG["w2_defer_f"]:
                    # defer the (large, phase-B-only) w2 load past startup
                    for ff in range(KF):
                        _split_dma(
                            w2eng,
                            w2_sb[:, ff * H : (ff + 1) * H],
                            w2d[:, ff * H : (ff + 1) * H],
                            CFG["w2_split"],
                        )
                for ci, (c0, cb) in enumerate(chunks):
                    if f == 0:
                        if ci == 0:
                            # startup-critical: first chunk's columns, finely split
                            for kk in range(KH):
                                _split_dma(
                                    xeng,
                                    x_sb[:, kk * C + c0 : kk * C + c0 + cb],
                                    xd[:, kk * C + c0 : kk * C + c0 + cb],
                                    CFG["x_split0"],
                                )
                        elif ci == 1:
                            # everything else in one go, ahead of the w2 bulk
                            for kk in range(KH):
                                _split_dma(
                                    xeng,
                                    x_sb[:, kk * C + c0 : kk * C + C],
                                    xd[:, kk * C + c0 : kk * C + C],
                                    CFG["x_split"],
                                )
                    ps1 = psum.tile([P, cb], FP32, tag="ps1", bufs=CFG["ps1_bufs"])
                    ps3 = psum.tile([P, cb], FP32, tag="ps3", bufs=CFG["ps1_bufs"])
                    for kk in range(KH):
                        nc.tensor.matmul(
                            ps1[:],
                            lhsT=w1f[:, kk * P : (kk + 1) * P],
                            rhs=x_sb[:, kk * C + c0 : kk * C + c0 + cb],
                            start=(kk == 0),
                            stop=(kk == KH - 1),
                        )
                    for kk in range(KH):
                        nc.tensor.matmul(
                            ps3[:],
                            lhsT=w3f[:, kk * P : (kk + 1) * P],
                            rhs=x_sb[:, kk * C + c0 : kk * C + c0 + cb],
                            start=(kk == 0),
                            stop=(kk == KH - 1),
                        )
                    tmp = pool.tile([P, cb], BF16_DT, tag="silu", bufs=CFG["silu_bufs"])
                    nc.scalar.activation(
                        tmp[:], ps1[:], mybir.ActivationFunctionType.Silu
                    )
                    nc.vector.tensor_mul(
                        act_sb[:, f * C + c0 : f * C + c0 + cb], tmp[:], ps3[:]
                    )

            # ---- phase B: y[c, h] = act.T @ w2, gate-scaled
            for ct in range(-(-C // P)):
                tp = min(P, C - ct * P)   # partial final token-tile
                for hb in range(HB):
                    ps2 = psum.tile([P, 512], FP32, tag="ps2", bufs=CFG["ps2_bufs"])
                    for f in range(KF):
                        nc.tensor.matmul(
                            ps2[:tp],
                            lhsT=act_sb[:, f * C + ct * P : f * C + ct * P + tp],
                            rhs=w2_sb[:, f * H + hb * 512 : f * H + (hb + 1) * 512],
                            start=(f == 0),
                            stop=(f == KF - 1),
                        )
                    o = pool.tile([P, 512], FP32, tag="o", bufs=CFG["o_bufs"])
                    if g_sb is not None:
                        nc.vector.tensor_scalar_mul(
                            o[:tp], ps2[:tp], g_sb[:tp, ct : ct + 1]
                        )
                    else:
                        nc.vector.tensor_copy(o[:tp], ps2[:tp])
                    _split_dma(
                        oeng,
                        yd[ct * P : ct * P + tp, hb * 512 : (hb + 1) * 512],
                        o[:tp],
                        CFG["out_split"],
                    )

        problem("r", dram["xr"].ap(), dram["w1"].ap(), dram["w3"].ap(),
                dram["w2"].ap(), yr.ap(), C_r, gd=dram["gr"].ap())
        problem("s", dram["xs"].ap(), dram["s1"].ap(), dram["s3"].ap(),
                dram["s2"].ap(), ys.ap(), C_s)

    return nc


_cache = {}


def _get_nc(C_r, C_s):
    key = (C_r, C_s, tuple(sorted(CFG.items())))
    if key not in _cache:
        nc = bacc.Bacc("TRN2", target_bir_lowering=False, debug=False,
                       num_devices=NCORES)
        _build(nc, C_r, C_s)
        nc.compile()
        _cache[key] = nc
    return _cache[key]


def _tile_w13(w):
    """[F, H] fp32 -> [128, KF*KH*128] bf16, (f, kk, j) column order."""
    a = np.ascontiguousarray(w, np.float32).astype(BF16)
    return np.ascontiguousarray(
        a.reshape(KF, P, KH, P).transpose(3, 0, 2, 1)
    ).reshape(P, KF * KH * P)


def _tile_w2(w):
    """[H, F] fp32 -> [128, KF*H] bf16, (f, h) column order."""
    a = np.ascontiguousarray(w, np.float32).astype(BF16)
    return np.ascontiguousarray(a.reshape(H, KF, P).transpose(2, 1, 0)).reshape(
        P, KF * H
    )


def _pad_rows(x, n):
    if x.shape[0] == n:
        return x
    out = np.zeros((n, x.shape[1]), x.dtype)
    out[: x.shape[0]] = x
    return out


def _tile_x(x):
    """[C, H] fp32 -> [128, KH*C] bf16, (kk, c) column order."""
    C = x.shape[0]
    a = x.astype(BF16)
    return np.ascontiguousarray(a.reshape(C, KH, P).transpose(2, 1, 0)).reshape(
        P, KH * C
    )


def kernel(hidden_states, gate_w, bias, ws1, ws2, ws3, we1, we2, we3):
    orig_shape = hidden_states.shape
    x = np.ascontiguousarray(
        np.asarray(hidden_states, np.float32).reshape(-1, orig_shape[-1])
    )
    T = x.shape[0]
    gate_w = np.asarray(gate_w, np.float32)
    bias = np.asarray(bias, np.float32)
    we1 = np.asarray(we1, np.float32)
    we2 = np.asarray(we2, np.float32)
    we3 = np.asarray(we3, np.float32)
    assert gate_w.shape[0] == E and we1.shape[0] == E and x.shape[1] == H

    # ---- host router (fp32, matches the reference's selection math)
    logits = x @ gate_w.T                                 # [T, E]
    scores = np.where(
        logits >= 0,
        1.0 / (1.0 + np.exp(-np.abs(logits))),
        1.0 - 1.0 / (1.0 + np.exp(-np.abs(logits))),
    ).astype(np.float32)
    routing = scores + bias[None, :]
    topk = np.argsort(-routing, axis=1, kind="stable")[:, :TOPK]  # [T, K]
    sel = np.take_along_axis(scores, topk, axis=1)
    gates = sel / sel.sum(axis=1, keepdims=True)          # [T, K]

    idx_e = []      # token ids routed to expert e
    gate_e = []     # matching combine weights
    for e in range(E):
        mask = topk == e                      # [T, K], at most one True per row
        rows = np.nonzero(mask.any(axis=1))[0]
        idx_e.append(rows)
        gate_e.append(gates[mask].astype(np.float32))  # row-major -> rows order

    max_n = max(len(r) for r in idx_e)
    C_r = max(64, -(-max_n // 64) * 64)   # routed capacity, multiple of 64
    C_s = max(64, -(-T // (NCORES * 64)) * 64)  # shared tokens per core

    nc = _get_nc(C_r, C_s)

    # ---- build per-core input maps
    shared_w = {
        "s1": _tile_w13(ws1),
        "s3": _tile_w13(ws3),
        "s2": _tile_w2(ws2),
    }
    in_maps = []
    for e in range(E):
        rows = idx_e[e]
        xg = np.zeros((C_r, H), np.float32)
        xg[: len(rows)] = x[rows]
        ctiles = -(-C_r // P)
        g = np.zeros((ctiles * P,), np.float32)
        g[: len(rows)] = gate_e[e]
        m = {
            "xr": _tile_x(xg),
            "gr": np.ascontiguousarray(g.reshape(ctiles, P).T),
            "w1": _tile_w13(we1[e]),
            "w3": _tile_w13(we3[e]),
            "w2": _tile_w2(we2[e]),
            "xs": _tile_x(_pad_rows(x[e * C_s : (e + 1) * C_s], C_s)),
        }
        m.update(shared_w)
        in_maps.append(m)

    res = run_bass_kernel_spmd(nc, in_maps, list(range(NCORES))).results

    # ---- host combine
    out = np.zeros((T, H), np.float32)
    for e in range(E):
        rows = idx_e[e]
        out[rows] += res[e]["yr"][: len(rows)]
        lo = e * C_s
        hi = min(T, (e + 1) * C_s)
        if lo < hi:
            out[lo:hi] += res[e]["ys"][: hi - lo]
    return out.reshape(orig_shape).astype(np.float32)
